# revision 6
# baseline (speedup 1.0000x reference)
"""Trainium2 Bass kernel for a diagonal SSM layer (B=8, S=4096, D=1024, N=4).

Math per batch row (one NeuronCore per batch row, data-parallel over batch):
    u  = x @ B_w.T + B_b                    (S, N)
    h_t = lam * h_{t-1} + u_t               (sequential scan, lam = sigmoid(log_lambda))
    y  = h @ C_w.T + C_b                    (S, D)
    out = LayerNorm(x + y) * ln_w + ln_b

Device mapping:
  - x streams in per 128-step chunk (natural layout: steps on partitions).
  - PE transposes x chunks (128x128 tiles) -> PSUM -> ACT copies to SBUF,
    giving x.T tiles (d on partitions) for the u-projection.
  - u.T (N, S) accumulates in PSUM via matmuls with B_w.T subtiles as the
    stationary operand; the PSUM->SBUF copy rides B_b in via the ACT bias.
  - The sequential scan runs on the DVE with tensor_tensor_scan
    (state = lam*state + u), chained across 512-step segments.
  - y + residual: rank-5 matmul (h rows 0..3 plus a ones row carrying C_b)
    accumulated on top of an identity matmul of x (residual) in PSUM, or a
    DVE add, depending on cfg.
  - LayerNorm: bn_stats/bn_aggr (DVE), rstd via ACT sqrt + DVE reciprocal,
    apply via ACT activation with per-partition scale/bias.
"""

import numpy as np

import concourse.bass as bass
import concourse.mybir as mybir
from concourse import tile
from concourse.bass_utils import run_bass_kernel_spmd

NB, S, D, NS = 8, 4096, 1024, 4
L = 128                 # chunk length (timesteps per partition block)
NCH = S // L            # 32 chunks
GRP = 4                 # chunks per group
NG = NCH // GRP         # 8 groups
SEG = GRP * L           # 512 timesteps per group segment
LN_EPS = 1e-5
F32 = mybir.dt.float32
F32R = mybir.dt.float32r
AF = mybir.ActivationFunctionType
ALU = mybir.AluOpType


def _split_excess_waits(nc, max_waits=1):
    """This toolchain's walrus accepts at most one sem-wait per instruction.
    Move extra waits onto preceding same-engine NOPs (engines are in-order,
    so an earlier wait on the same engine is equivalent)."""
    moved = 0
    for f in nc.m.functions:
        for bb in f.blocks:
            out = []
            for inst in bb.instructions:
                si = inst.sync_info
                if si is not None and len(si.on_wait) > max_waits:
                    waits = list(si.on_wait)
                    for w in waits[:-max_waits]:
                        nop = mybir.InstNoOp(
                            name=f"WSPL-{nc.next_id()}", ins=[], outs=[])
                        nop.engine = inst.engine
                        nop.sync_info = mybir.SyncInfo(on_wait=[w], on_update=[])
                        out.append(nop)
                        moved += 1
                    inst.sync_info = mybir.SyncInfo(
                        on_wait=waits[-max_waits:], on_update=list(si.on_update))
                if si is not None and len(si.on_update) > 1:
                    raise RuntimeError(
                        f"instruction {inst.name} has {len(si.on_update)} updates")
                out.append(inst)
            bb.instructions = out
    return moved


class Cfg:
    def __init__(self, u_f32r=False, y_f32r=False, tr_f32r=False,
                 resid_pe=False, resid_f32r=False, ln_affine=False):
        self.u_f32r = u_f32r          # float32r for the u-projection matmuls
        self.y_f32r = y_f32r          # float32r for the y matmuls
        self.tr_f32r = tr_f32r        # float32r transpose mode
        self.resid_pe = resid_pe      # residual add via identity matmul on PE
        self.resid_f32r = resid_f32r  # float32r for the identity matmul
        self.ln_affine = ln_affine    # emit ln_w/ln_b application
        self.key = (u_f32r, y_f32r, tr_f32r, resid_pe, resid_f32r, ln_affine)


def _r(ap, flag):
    return ap.bitcast(F32R) if flag else ap


def build_nc(cfg: Cfg):
    nc = bass.Bass("TRN2", target_bir_lowering=False, debug=False)

    x_d = nc.dram_tensor("x", [S, D], F32, kind="ExternalInput")
    bwt_d = nc.dram_tensor("bwt", [128, 8 * NS], F32, kind="ExternalInput")
    lamb_d = nc.dram_tensor("lamb", [NS, SEG], F32, kind="ExternalInput")
    bb_d = nc.dram_tensor("bb", [NS, 1], F32, kind="ExternalInput")
    cwa_d = nc.dram_tensor("cwa", [NS + 1, D], F32, kind="ExternalInput")
    idn_d = nc.dram_tensor("idn", [128, 128], F32, kind="ExternalInput")
    ones_d = nc.dram_tensor("ones", [1, S], F32, kind="ExternalInput")
    out_d = nc.dram_tensor("out", [S, D], F32, kind="ExternalOutput")
    if cfg.ln_affine:
        lnw_d = nc.dram_tensor("lnw", [128, D], F32, kind="ExternalInput")
        lnb_d = nc.dram_tensor("lnb", [128, D], F32, kind="ExternalInput")

    with tile.TileContext(nc) as tc:
        with (
            tc.tile_pool(name="consts", bufs=1) as cpool,
            tc.tile_pool(name="xin", bufs=NCH) as xpool,
            tc.tile_pool(name="xT", bufs=3) as xtp,
            tc.tile_pool(name="uT", bufs=3) as utp,
            tc.tile_pool(name="hseq", bufs=1) as hsp,
            tc.tile_pool(name="zs", bufs=3) as zsp,
            tc.tile_pool(name="stats", bufs=4) as stp,
            tc.tile_pool(name="outb", bufs=3) as obp,
            tc.tile_pool(name="ps_tr", bufs=2, space="PSUM") as ptr,
            tc.tile_pool(name="ps_u", bufs=2, space="PSUM") as pup,
            tc.tile_pool(name="ps_z", bufs=2, space="PSUM") as pzp,
        ):
            bwt = cpool.tile([128, 8 * NS], F32)
            nc.sync.dma_start(bwt[:], bwt_d[:])
            lamb = cpool.tile([NS, SEG], F32)
            nc.sync.dma_start(lamb[:], lamb_d[:])
            bb = cpool.tile([NS, 1], F32)
            nc.sync.dma_start(bb[:], bb_d[:])
            cwa = cpool.tile([NS + 1, D], F32)
            nc.sync.dma_start(cwa[:], cwa_d[:])
            idn = cpool.tile([128, 128], F32)
            nc.sync.dma_start(idn[:], idn_d[:])
            if cfg.ln_affine:
                lnw = cpool.tile([128, D], F32)
                nc.sync.dma_start(lnw[:], lnw_d[:])
                lnb = cpool.tile([128, D], F32)
                nc.sync.dma_start(lnb[:], lnb_d[:])

            # h sequence (rows 0..3) plus a constant ones row (carries C_b
            # through the rank-5 y matmul). Static tile; row 4 loaded once.
            hs = hsp.tile([NS + 1, S], F32)
            nc.sync.dma_start(hs[NS:NS + 1, :], ones_d[:])

            xc = []
            for c in range(NCH):
                t = xpool.tile([128, D], F32, tag="xc")
                nc.sync.dma_start(t[:], x_d[c * L:(c + 1) * L, :])
                xc.append(t)

            for g in range(NG):
                # ---- Phase A: transpose 4 chunks, project to u.T ----
                pu = pup.tile([NS, SEG], F32, tag="pu")
                for j in range(8):
                    pt = ptr.tile([128, SEG], F32, tag="pt")
                    for ci in range(GRP):
                        c = g * GRP + ci
                        nc.tensor.transpose(
                            pt[:, ci * L:(ci + 1) * L],
                            _r(xc[c][:, j * 128:(j + 1) * 128], cfg.tr_f32r),
                            _r(idn[:], cfg.tr_f32r),
                        )
                    xt = xtp.tile([128, SEG], F32, tag="xt")
                    nc.scalar.copy(xt[:], pt[:])
                    nc.tensor.matmul(
                        pu[:],
                        lhsT=_r(bwt[:, j * NS:(j + 1) * NS], cfg.u_f32r),
                        rhs=_r(xt[:], cfg.u_f32r),
                        start=(j == 0), stop=(j == 7),
                    )
                # PSUM -> SBUF with B_b folded in via the ACT bias
                uT = utp.tile([NS, SEG], F32, tag="uT")
                nc.scalar.activation(uT[:], pu[:], AF.Identity,
                                     bias=bb[:], scale=1.0)

                # ---- sequential scan: h = lam*h + u, chained across groups
                seg = slice(g * SEG, (g + 1) * SEG)
                init = 0.0 if g == 0 else hs[0:NS, g * SEG - 1:g * SEG]
                nc.vector.tensor_tensor_scan(
                    hs[0:NS, seg], lamb[:], uT[:],
                    initial=init, op0=ALU.mult, op1=ALU.add)

                # ---- Phase B: y matmul + residual + layernorm per chunk ----
                for ci in range(GRP):
                    c = g * GRP + ci
                    z = pzp.tile([128, D], F32, tag="z")
                    lhs_y = hs[:, c * L:(c + 1) * L]
                    for h in range(2):
                        cols = slice(h * 512, (h + 1) * 512)
                        if cfg.resid_pe:
                            nc.tensor.matmul(
                                z[:, cols],
                                lhsT=_r(idn[:], cfg.resid_f32r),
                                rhs=_r(xc[c][:, cols], cfg.resid_f32r),
                                start=True, stop=False)
                            nc.tensor.matmul(
                                z[:, cols],
                                lhsT=_r(lhs_y, cfg.y_f32r),
                                rhs=_r(cwa[:, cols], cfg.y_f32r),
                                start=False, stop=True)
                        else:
                            nc.tensor.matmul(
                                z[:, cols],
                                lhsT=_r(lhs_y, cfg.y_f32r),
                                rhs=_r(cwa[:, cols], cfg.y_f32r),
                                start=True, stop=True)
                    if cfg.resid_pe:
                        zsrc = z
                    else:
                        zsb = zsp.tile([128, D], F32, tag="zsb")
                        nc.vector.tensor_tensor(zsb[:], xc[c][:], z[:], ALU.add)
                        zsrc = zsb

                    st = stp.tile([128, 12], F32, tag="st")
                    nc.vector.bn_stats(st[:, 0:6], zsrc[:, 0:512])
                    nc.vector.bn_stats(st[:, 6:12], zsrc[:, 512:1024])
                    mv = stp.tile([128, 2], F32, tag="mv")
                    nc.vector.bn_aggr(mv[:], st[:])
                    veps = stp.tile([128, 1], F32, tag="veps")
                    nc.vector.tensor_scalar_add(veps[:], mv[:, 1:2], LN_EPS)
                    std = stp.tile([128, 1], F32, tag="std")
                    nc.scalar.sqrt(std[:], veps[:])
                    rstd = stp.tile([128, 1], F32, tag="rstd")
                    nc.vector.reciprocal(rstd[:], std[:])
                    nmr = stp.tile([128, 1], F32, tag="nmr")
                    nc.vector.tensor_scalar(
                        nmr[:], mv[:, 0:1], rstd[:], -1.0,
                        ALU.mult, ALU.mult)

                    ob = obp.tile([128, D], F32, tag="ob")
                    nc.scalar.activation(ob[:], zsrc[:], AF.Identity,
                                         bias=nmr[:], scale=rstd[:])
                    if cfg.ln_affine:
                        nc.vector.tensor_tensor(ob[:], ob[:], lnw[:], ALU.mult)
                        nc.vector.tensor_tensor(ob[:], ob[:], lnb[:], ALU.add)
                    nc.sync.dma_start(out_d[c * L:(c + 1) * L, :], ob[:])

    _split_excess_waits(nc)
    return nc


_NC_CACHE = {}


def _get_nc(cfg: Cfg):
    if cfg.key not in _NC_CACHE:
        _NC_CACHE[cfg.key] = build_nc(cfg)
    return _NC_CACHE[cfg.key]


DEFAULT_CFG = Cfg()


def make_inputs(x, log_lambda, B_w, B_b, C_w, C_b, ln_w, ln_b, cfg):
    lam = (1.0 / (1.0 + np.exp(-np.float64(log_lambda)))).astype(np.float32)
    # bwt[p, 4j+n] = B_w[n, 128j+p]
    bwt = np.ascontiguousarray(
        np.transpose(np.asarray(B_w, np.float32).T.reshape(8, 128, NS),
                     (1, 0, 2)).reshape(128, 8 * NS))
    lamb = np.ascontiguousarray(np.tile(lam[:, None], (1, SEG)))
    bb = np.ascontiguousarray(np.asarray(B_b, np.float32)[:, None])
    cwa = np.ascontiguousarray(
        np.concatenate([np.asarray(C_w, np.float32).T,
                        np.asarray(C_b, np.float32)[None, :]], 0))
    idn = np.eye(128, dtype=np.float32)
    ones = np.ones((1, S), np.float32)
    shared = {"bwt": bwt, "lamb": lamb, "bb": bb, "cwa": cwa, "idn": idn,
              "ones": ones}
    if cfg.ln_affine:
        shared["lnw"] = np.ascontiguousarray(
            np.tile(np.asarray(ln_w, np.float32)[None, :], (128, 1)))
        shared["lnb"] = np.ascontiguousarray(
            np.tile(np.asarray(ln_b, np.float32)[None, :], (128, 1)))
    x = np.asarray(x, np.float32)
    return [dict(shared, x=np.ascontiguousarray(x[b])) for b in range(NB)]


def run(inputs, cfg=None, **spmd_kwargs):
    cfg = cfg or DEFAULT_CFG
    ln_w = np.asarray(inputs["ln_w"], np.float32)
    ln_b = np.asarray(inputs["ln_b"], np.float32)
    affine = not (np.allclose(ln_w, 1.0) and np.allclose(ln_b, 0.0))
    if affine != cfg.ln_affine:
        cfg = Cfg(*cfg.key[:5], ln_affine=affine)
    nc = _get_nc(cfg)
    in_maps = make_inputs(
        inputs["x"], inputs["log_lambda"], inputs["B_w"], inputs["B_b"],
        inputs["C_w"], inputs["C_b"], ln_w, ln_b, cfg)
    res = run_bass_kernel_spmd(nc, in_maps, core_ids=list(range(NB)),
                               **spmd_kwargs)
    out = np.stack([res.results[b]["out"] for b in range(NB)], 0)
    return out, res


def kernel(**inputs):
    out, _ = run(inputs, DEFAULT_CFG)
    return out


# revision 30
# speedup vs baseline: 681.3007x; 681.3007x over previous
"""Trainium2 Bass kernel for a diagonal SSM layer (B=8, S=4096, D=1024, N=4).

Math per batch row (one NeuronCore per batch row, data-parallel over batch):
    u  = x @ B_w.T + B_b                    (S, N)
    h_t = lam * h_{t-1} + u_t               (sequential scan, lam = sigmoid(log_lambda))
    y  = h @ C_w.T + C_b                    (S, D)
    out = LayerNorm(x + y) * ln_w + ln_b

Device mapping:
  - x streams in per 128-step chunk (natural layout: steps on partitions).
  - PE transposes x chunks (128x128 tiles) -> PSUM -> ACT copies to SBUF,
    giving x.T tiles (d on partitions) for the u-projection.
  - u.T (N, S) accumulates in PSUM via matmuls with B_w.T subtiles as the
    stationary operand; the PSUM->SBUF copy rides B_b in via the ACT bias.
  - The sequential scan runs on the DVE with tensor_tensor_scan
    (state = lam*state + u), chained across 512-step segments.
  - y + residual: rank-5 matmul (h rows 0..3 plus a ones row carrying C_b)
    accumulated on top of an identity matmul of x (residual) in PSUM, or a
    DVE add, depending on cfg.
  - LayerNorm: bn_stats/bn_aggr (DVE), rstd via ACT sqrt + DVE reciprocal,
    apply via ACT activation with per-partition scale/bias.
"""

import numpy as np

import concourse.bass as bass
import concourse.mybir as mybir
from concourse import tile
from concourse.bass_utils import run_bass_kernel_spmd

NB, S, D, NS = 8, 4096, 1024, 4
L = 128                 # chunk length (timesteps per partition block)
NCH = S // L            # 32 chunks
GRP = 4                 # chunks per group
NG = NCH // GRP         # 8 groups
SEG = GRP * L           # 512 timesteps per group segment
LN_EPS = 1e-5
F32 = mybir.dt.float32
F32R = mybir.dt.float32r
AF = mybir.ActivationFunctionType
ALU = mybir.AluOpType


def _split_excess_waits(nc, max_waits=1):
    """This toolchain's walrus accepts at most one sem-wait per instruction.
    Move extra waits onto preceding same-engine NOPs (engines are in-order,
    so an earlier wait on the same engine is equivalent)."""
    moved = 0
    for f in nc.m.functions:
        for bb in f.blocks:
            out = []
            for inst in bb.instructions:
                si = inst.sync_info
                if si is not None and len(si.on_wait) > max_waits:
                    waits = list(si.on_wait)
                    for w in waits[:-max_waits]:
                        nop = mybir.InstNoOp(
                            name=f"WSPL-{nc.next_id()}", ins=[], outs=[])
                        nop.engine = inst.engine
                        nop.sync_info = mybir.SyncInfo(on_wait=[w], on_update=[])
                        out.append(nop)
                        moved += 1
                    inst.sync_info = mybir.SyncInfo(
                        on_wait=waits[-max_waits:], on_update=list(si.on_update))
                if si is not None and len(si.on_update) > 1:
                    raise RuntimeError(
                        f"instruction {inst.name} has {len(si.on_update)} updates")
                out.append(inst)
            bb.instructions = out
    return moved


class Cfg:
    def __init__(self, u_f32r=False, y_f32r=False, tr_f32r=False,
                 resid_pe=False, resid_f32r=False, ln_affine=False,
                 xbufs=32, zbufs=3, obufs=3, xtbufs=3, utbufs=3,
                 ptrbufs=2, pubufs=2, pzbufs=2, cb_dve_mod=-1,
                 groups=(4,) * 8, pair_dma=False, u_natural=False,
                 putbufs=1, tiny_gpsimd=False):
        self.u_f32r = u_f32r          # float32r for the u-projection matmuls
        self.y_f32r = y_f32r          # float32r for the y matmuls
        self.tr_f32r = tr_f32r        # float32r transpose mode
        self.resid_pe = resid_pe      # residual add via identity matmul on PE
        self.resid_f32r = resid_f32r  # float32r for the identity matmul
        self.ln_affine = ln_affine    # emit ln_w/ln_b application
        self.xbufs = xbufs
        self.zbufs = zbufs
        self.obufs = obufs
        self.xtbufs = xtbufs
        self.utbufs = utbufs
        self.ptrbufs = ptrbufs
        self.pubufs = pubufs
        self.pzbufs = pzbufs
        self.cb_dve_mod = cb_dve_mod  # j%4 == this -> transpose copyback on DVE
        self.groups = tuple(groups)   # chunks per scan segment, sums to 32
        assert sum(self.groups) == NCH
        self.pair_dma = pair_dma      # 2 chunks (1 MiB) per in/out DMA
        # u matmul orientation: stationary x.T tiles producing u natural
        # (cheap N=4 fp32 matmuls), then a tiny transpose to u.T.
        self.u_natural = u_natural
        self.putbufs = putbufs
        self.tiny_gpsimd = tiny_gpsimd
        self.key = (u_f32r, y_f32r, tr_f32r, resid_pe, resid_f32r, ln_affine,
                    xbufs, zbufs, obufs, xtbufs, utbufs,
                    ptrbufs, pubufs, pzbufs, cb_dve_mod,
                    self.groups, pair_dma, u_natural, putbufs, tiny_gpsimd)


def _r(ap, flag):
    return ap.bitcast(F32R) if flag else ap


def build_nc(cfg: Cfg):
    nc = bass.Bass("TRN2", target_bir_lowering=False, debug=False)

    # float32r is a reduced-precision PE format: any SBUF operand of an
    # f32r matmul must itself be written as f32r (verifier-enforced), so
    # the dtype is threaded through tiles and DRAM decls rather than cast.
    if cfg.tr_f32r:
        assert cfg.u_f32r, "f32r transposes feed the u matmul; enable u_f32r"
    if cfg.u_natural:
        assert not (cfg.u_f32r or cfg.tr_f32r)
    XDT = F32R if cfg.tr_f32r else F32
    UDT = F32R if cfg.u_f32r else F32
    YDT = F32R if cfg.y_f32r else F32

    x_d = nc.dram_tensor("x", [S, D], XDT, kind="ExternalInput")
    bwt_d = nc.dram_tensor("bwt", [128, 8 * NS], UDT, kind="ExternalInput")
    lamb_d = nc.dram_tensor("lamb", [NS, SEG], F32, kind="ExternalInput")
    bb_d = nc.dram_tensor("bb", [NS, 1], F32, kind="ExternalInput")
    cwa_d = nc.dram_tensor("cwa", [NS + 1, D], YDT, kind="ExternalInput")
    idn_d = nc.dram_tensor("idn", [128, 128], XDT, kind="ExternalInput")
    ones_d = nc.dram_tensor("ones", [1, S], YDT, kind="ExternalInput")
    out_d = nc.dram_tensor("out", [S, D], F32, kind="ExternalOutput")
    if cfg.ln_affine:
        lnw_d = nc.dram_tensor("lnw", [128, D], F32, kind="ExternalInput")
        lnb_d = nc.dram_tensor("lnb", [128, D], F32, kind="ExternalInput")

    with tile.TileContext(nc) as tc:
        with (
            tc.tile_pool(name="consts", bufs=1) as cpool,
            tc.tile_pool(name="xin", bufs=cfg.xbufs) as xpool,
            tc.tile_pool(name="xT", bufs=cfg.xtbufs) as xtp,
            tc.tile_pool(name="uT", bufs=cfg.utbufs) as utp,
            tc.tile_pool(name="hseq", bufs=1) as hsp,
            tc.tile_pool(name="zs", bufs=cfg.zbufs) as zsp,
            tc.tile_pool(name="stats", bufs=6) as stp,
            tc.tile_pool(name="outb", bufs=cfg.obufs) as obp,
            tc.tile_pool(name="ps_tr", bufs=cfg.ptrbufs, space="PSUM") as ptr,
            tc.tile_pool(name="ps_u", bufs=cfg.pubufs, space="PSUM") as pup,
            tc.tile_pool(name="ps_ut", bufs=cfg.putbufs, space="PSUM") as put_p,
            tc.tile_pool(name="ps_z", bufs=cfg.pzbufs, space="PSUM") as pzp,
        ):
            bwt = cpool.tile([128, 8 * NS], UDT)
            nc.sync.dma_start(bwt[:], bwt_d[:])
            lamb = cpool.tile([NS, SEG], F32)
            nc.sync.dma_start(lamb[:], lamb_d[:])
            bb = cpool.tile([NS, 1], F32)
            nc.sync.dma_start(bb[:], bb_d[:])
            cwa = cpool.tile([NS + 1, D], YDT)
            nc.sync.dma_start(cwa[:], cwa_d[:])
            idn = cpool.tile([128, 128], XDT)
            nc.sync.dma_start(idn[:], idn_d[:])
            if cfg.ln_affine:
                lnw = cpool.tile([128, D], F32)
                nc.sync.dma_start(lnw[:], lnw_d[:])
                lnb = cpool.tile([128, D], F32)
                nc.sync.dma_start(lnb[:], lnb_d[:])

            # h sequence (rows 0..3) plus a constant ones row (carries C_b
            # through the rank-5 y matmul). Static tile; row 4 loaded once.
            hs = hsp.tile([NS + 1, S], YDT)
            nc.sync.dma_start(hs[NS:NS + 1, :], ones_d[:])

            # x chunk tiles: one chunk (512 KiB) or two (1 MiB) per DMA
            if cfg.pair_dma:
                xpair = []
                for p in range(NCH // 2):
                    t = xpool.tile([128, 2 * D], XDT, tag="xc")
                    nc.sync.dma_start(
                        t[:].rearrange("p (c d) -> p c d", c=2),
                        x_d[p * 2 * L:(p + 1) * 2 * L, :].rearrange(
                            "(c p) d -> p c d", p=128))
                    xpair.append(t)

                def xap(c):
                    return xpair[c // 2][:, (c % 2) * D:(c % 2 + 1) * D]
            else:
                xc = []
                for c in range(NCH):
                    t = xpool.tile([128, D], XDT, tag="xc")
                    nc.sync.dma_start(t[:], x_d[c * L:(c + 1) * L, :])
                    xc.append(t)

                def xap(c):
                    return xc[c][:]

            ob_t = None  # paired output staging
            c0g = 0      # first chunk of current group
            for gsz in cfg.groups:
                glen = gsz * L
                # ---- Phase A: transpose gsz chunks, project to u.T ----
                if cfg.u_natural:
                    uT = utp.tile([NS, SEG], F32, tag="uT")
                    for ci in range(gsz):
                        c = c0g + ci
                        xt = xtp.tile([128, 8 * L], F32, tag="xt")
                        for jh in range(2):
                            pt = ptr.tile([128, SEG], XDT, tag="pt")
                            for j4 in range(4):
                                j = jh * 4 + j4
                                nc.tensor.transpose(
                                    pt[:, j4 * L:(j4 + 1) * L],
                                    xap(c)[:, j * 128:(j + 1) * 128],
                                    idn[:])
                            nc.scalar.copy(xt[:, jh * SEG:(jh + 1) * SEG], pt[:])
                        pnat = pup.tile([128, NS], F32, tag="pu")
                        for j in range(8):
                            nc.tensor.matmul(
                                pnat[:],
                                lhsT=xt[:, j * L:(j + 1) * L],
                                rhs=bwt[:, j * NS:(j + 1) * NS],
                                start=(j == 0), stop=(j == 7))
                        usb = stp.tile([128, NS], F32, tag="usb")
                        nc.vector.tensor_copy(usb[:], pnat[:])
                        putt = put_p.tile([NS, L], F32, tag="put")
                        nc.tensor.transpose(putt[:], usb[:], idn[:])
                        nc.scalar.activation(
                            uT[:, ci * L:(ci + 1) * L], putt[:],
                            AF.Identity, bias=bb[:], scale=1.0)
                    t0 = c0g * L
                    seg = slice(t0, t0 + glen)
                    init = 0.0 if c0g == 0 else hs[0:NS, t0 - 1:t0]
                    nc.vector.tensor_tensor_scan(
                        hs[0:NS, seg], lamb[:, :glen], uT[:, :glen],
                        initial=init, op0=ALU.mult, op1=ALU.add)
                else:
                    pu = pup.tile([NS, SEG], F32, tag="pu")
                    for j in range(8):
                        pt = ptr.tile([128, SEG], XDT, tag="pt")
                        for ci in range(gsz):
                            c = c0g + ci
                            nc.tensor.transpose(
                                pt[:, ci * L:(ci + 1) * L],
                                xap(c)[:, j * 128:(j + 1) * 128],
                                idn[:],
                            )
                        xt = xtp.tile([128, SEG], UDT, tag="xt")
                        if j % 4 == cfg.cb_dve_mod:
                            nc.vector.tensor_copy(xt[:, :glen], pt[:, :glen])
                        else:
                            nc.scalar.copy(xt[:, :glen], pt[:, :glen])
                        nc.tensor.matmul(
                            pu[:, :glen],
                            lhsT=bwt[:, j * NS:(j + 1) * NS],
                            rhs=xt[:, :glen],
                            start=(j == 0), stop=(j == 7),
                        )
                    # PSUM -> SBUF with B_b folded in via the ACT bias
                    uT = utp.tile([NS, SEG], F32, tag="uT")
                    nc.scalar.activation(uT[:, :glen], pu[:, :glen],
                                         AF.Identity, bias=bb[:], scale=1.0)

                    # -- sequential scan: h = lam*h + u, chained across groups
                    t0 = c0g * L
                    seg = slice(t0, t0 + glen)
                    init = 0.0 if c0g == 0 else hs[0:NS, t0 - 1:t0]
                    nc.vector.tensor_tensor_scan(
                        hs[0:NS, seg], lamb[:, :glen], uT[:, :glen],
                        initial=init, op0=ALU.mult, op1=ALU.add)

                # ---- Phase B: y matmul + residual + layernorm per chunk ----
                for ci in range(gsz):
                    c = c0g + ci
                    z = pzp.tile([128, D], F32, tag="z")
                    lhs_y = hs[:, c * L:(c + 1) * L]
                    for h in range(2):
                        cols = slice(h * 512, (h + 1) * 512)
                        if cfg.resid_pe:
                            nc.tensor.matmul(
                                z[:, cols],
                                lhsT=_r(idn[:], cfg.resid_f32r),
                                rhs=_r(xap(c)[:, cols], cfg.resid_f32r),
                                start=True, stop=False)
                            nc.tensor.matmul(
                                z[:, cols],
                                lhsT=lhs_y,
                                rhs=cwa[:, cols],
                                start=False, stop=True)
                        else:
                            nc.tensor.matmul(
                                z[:, cols],
                                lhsT=lhs_y,
                                rhs=cwa[:, cols],
                                start=True, stop=True)
                    if cfg.resid_pe:
                        zsrc = z
                    else:
                        zsb = zsp.tile([128, D], F32, tag="zsb")
                        xin = xap(c).bitcast(F32) if cfg.tr_f32r else xap(c)
                        nc.vector.tensor_tensor(zsb[:], xin, z[:], ALU.add)
                        zsrc = zsb

                    st = stp.tile([128, 12], F32, tag="st")
                    nc.vector.bn_stats(st[:, 0:6], zsrc[:, 0:512])
                    nc.vector.bn_stats(st[:, 6:12], zsrc[:, 512:1024])
                    mv = stp.tile([128, 2], F32, tag="mv")
                    nc.vector.bn_aggr(mv[:], st[:])
                    tiny = nc.gpsimd if cfg.tiny_gpsimd else nc.vector
                    veps = stp.tile([128, 1], F32, tag="veps")
                    tiny.tensor_scalar_add(veps[:], mv[:, 1:2], LN_EPS)
                    std = stp.tile([128, 1], F32, tag="std")
                    nc.scalar.sqrt(std[:], veps[:])
                    rstd = stp.tile([128, 1], F32, tag="rstd")
                    nc.vector.reciprocal(rstd[:], std[:])
                    nmr = stp.tile([128, 1], F32, tag="nmr")
                    tiny.tensor_scalar(
                        nmr[:], mv[:, 0:1], rstd[:], -1.0,
                        ALU.mult, ALU.mult)

                    if cfg.pair_dma:
                        if c % 2 == 0:
                            ob_t = obp.tile([128, 2 * D], F32, tag="ob")
                        oap = ob_t[:, (c % 2) * D:(c % 2 + 1) * D]
                    else:
                        ob_t = obp.tile([128, D], F32, tag="ob")
                        oap = ob_t[:]
                    nc.scalar.activation(oap, zsrc[:], AF.Identity,
                                         bias=nmr[:], scale=rstd[:])
                    if cfg.ln_affine:
                        nc.vector.tensor_tensor(oap, oap, lnw[:], ALU.mult)
                        nc.vector.tensor_tensor(oap, oap, lnb[:], ALU.add)
                    if cfg.pair_dma:
                        if c % 2 == 1:
                            p = c // 2
                            nc.sync.dma_start(
                                out_d[p * 2 * L:(p + 1) * 2 * L, :].rearrange(
                                    "(c p) d -> p c d", p=128),
                                ob_t[:].rearrange("p (c d) -> p c d", c=2))
                    else:
                        nc.sync.dma_start(out_d[c * L:(c + 1) * L, :], ob_t[:])
                c0g += gsz

    _split_excess_waits(nc)
    return nc


_NC_CACHE = {}


def _get_nc(cfg: Cfg):
    if cfg.key not in _NC_CACHE:
        _NC_CACHE[cfg.key] = build_nc(cfg)
    return _NC_CACHE[cfg.key]


# Best known-exact configuration (hardware-validated, rel err ~8.9e-7 vs the
# fp32 jax reference; cost-model estimate ~129 us/core vs ~94 us memory
# roofline). float32r variants are ~10% faster but cost ~1.4e-4 rel err.
DEFAULT_CFG = Cfg(u_natural=True, groups=(1,) * NCH,
                  xbufs=12, zbufs=8, obufs=8, xtbufs=6, utbufs=4,
                  pubufs=2, putbufs=2, pzbufs=1, tiny_gpsimd=True)


def make_inputs(x, log_lambda, B_w, B_b, C_w, C_b, ln_w, ln_b, cfg):
    lam = (1.0 / (1.0 + np.exp(-np.float64(log_lambda)))).astype(np.float32)
    # bwt[p, 4j+n] = B_w[n, 128j+p]
    bwt = np.ascontiguousarray(
        np.transpose(np.asarray(B_w, np.float32).T.reshape(8, 128, NS),
                     (1, 0, 2)).reshape(128, 8 * NS))
    lamb = np.ascontiguousarray(np.tile(lam[:, None], (1, SEG)))
    bb = np.ascontiguousarray(np.asarray(B_b, np.float32)[:, None])
    cwa = np.ascontiguousarray(
        np.concatenate([np.asarray(C_w, np.float32).T,
                        np.asarray(C_b, np.float32)[None, :]], 0))
    idn = np.eye(128, dtype=np.float32)
    ones = np.ones((1, S), np.float32)
    shared = {"bwt": bwt, "lamb": lamb, "bb": bb, "cwa": cwa, "idn": idn,
              "ones": ones}
    if cfg.ln_affine:
        shared["lnw"] = np.ascontiguousarray(
            np.tile(np.asarray(ln_w, np.float32)[None, :], (128, 1)))
        shared["lnb"] = np.ascontiguousarray(
            np.tile(np.asarray(ln_b, np.float32)[None, :], (128, 1)))
    x = np.asarray(x, np.float32)
    return [dict(shared, x=np.ascontiguousarray(x[b])) for b in range(NB)]


def run(inputs, cfg=None, **spmd_kwargs):
    cfg = cfg or DEFAULT_CFG
    ln_w = np.asarray(inputs["ln_w"], np.float32)
    ln_b = np.asarray(inputs["ln_b"], np.float32)
    affine = not (np.allclose(ln_w, 1.0) and np.allclose(ln_b, 0.0))
    if affine != cfg.ln_affine:
        cfg = Cfg(*cfg.key[:5], ln_affine=affine)
    nc = _get_nc(cfg)
    in_maps = make_inputs(
        inputs["x"], inputs["log_lambda"], inputs["B_w"], inputs["B_b"],
        inputs["C_w"], inputs["C_b"], ln_w, ln_b, cfg)
    res = run_bass_kernel_spmd(nc, in_maps, core_ids=list(range(NB)),
                               **spmd_kwargs)
    out = np.stack([res.results[b]["out"] for b in range(NB)], 0)
    return out, res


def kernel(**inputs):
    out, _ = run(inputs, DEFAULT_CFG)
    return out
